# revision 1
# baseline (speedup 1.0000x reference)
"""GCN layer (dense projection + sparse neighbor aggregation) on 8 Trainium2
NeuronCores via Bass/Tile.

Strategy: shard nodes (and their incident edges, grouped by destination row)
across the 8 cores; replicate W/b; AllGather the projected node features so
every core can gather arbitrary source columns (split into 4 group-aligned
quarter-collectives pipelined with the projection); per 128-row output block,
bulk-gather the needed source rows with DMAGatherAnt (int16 indices into 4
sub-tables of <=32k rows), scale by edge_val, and segment-sum via an
assignment-matrix matmul accumulated in PSUM (bias folded in as an extra
rank-128 matmul; padded gather slots are killed by rowloc=-1).
"""

import sys

if "/opt/trn_rl_repo" not in sys.path:
    sys.path.insert(0, "/opt/trn_rl_repo")

import numpy as np

import concourse.bass as bass
import concourse.mybir as mybir
import concourse.tile as tile
from concourse import bacc
from concourse.bass_utils import run_bass_kernel_spmd

N_NODES = 100000
N_EDGES = 1600000
IN_FT = 256
OUT_FT = 64
NCORES = 8
NS = N_NODES // NCORES          # 12500 nodes per core
NB = (NS + 127) // 128          # 98 row blocks per core
NSP = NB * 128                  # 12544 padded nodes per core
GB = 7                          # row blocks per pipeline group (98 = 14 * 7)
NGROUPS = NB // GB              # 14
QGROUPS = [2, 2, 2, 2, 2, 2, 2]  # groups per sub-collective (sums to 14)
NSUB = len(QGROUPS)

F32 = mybir.dt.float32
F16 = mybir.dt.float16
I32 = mybir.dt.int32
I16 = mybir.dt.int16

MAXCH = 8                       # 1024 indices = HW cap per dma_gather
NQ = 4                          # SWDGE queues (set to 1 for CoreSim runs)


def _quarter_layout():
    qg_end = np.cumsum(QGROUPS)          # groups per quarter, cumulative
    qb = [g * GB for g in QGROUPS]       # blocks per quarter
    qb_start = np.concatenate([[0], np.cumsum(qb)])
    subrows = [NCORES * 128 * n for n in qb]
    return qg_end, qb, qb_start, subrows


def build_program(nchb):
    """One SPMD Bass program; all 8 cores run it on their own shards.

    nchb[b]: 128-edge chunks per (row-block, bucket b).
    """
    nchb = list(nchb)
    ncht = sum(nchb)                    # chunks per row block
    off_b = np.concatenate([[0], np.cumsum(nchb)])  # chunk offset per bucket
    qg_end, qb, qb_start, subrows = _quarter_layout()
    # per-(group,bucket) stream: GB*nchb[b] chunks; xg region offsets
    reg_b = np.concatenate([[0], np.cumsum([GB * c for c in nchb])])
    sgt = int(reg_b[-1])                # chunks per group in xg
    # gidx column offsets per bucket (16-wrapped: 8 int16 cols per chunk)
    gcol_b = np.concatenate([[0], np.cumsum([GB * c * 8 for c in nchb])])
    gcols = int(gcol_b[-1])

    nq = NQ
    nc = bacc.Bacc("TRN2", target_bir_lowering=False, debug=False,
                   num_devices=NCORES, num_swdge_queues=nq)

    seqT = nc.dram_tensor("seqT", [2, 128, NSP], F32, kind="ExternalInput")
    gidx = nc.dram_tensor("gidx", [128, NGROUPS, gcols], I16,
                          kind="ExternalInput")
    val = nc.dram_tensor("val", [128, NGROUPS, sgt], F16,
                         kind="ExternalInput")
    rl = nc.dram_tensor("rl", [128, NB, ncht], F16, kind="ExternalInput")
    w_in = nc.dram_tensor("w", [128, 2, OUT_FT], F32, kind="ExternalInput")
    bias_in = nc.dram_tensor("biasb", [128, OUT_FT], F16,
                             kind="ExternalInput")
    # partition-major layouts: [p, block, feature]; host un-permutes
    sf_out = nc.dram_tensor("sf", [128, NB, OUT_FT], F32,
                            kind="ExternalOutput")
    agg_out = nc.dram_tensor("agg", [128, NB, OUT_FT], F32,
                             kind="ExternalOutput")
    ccin = [nc.dram_tensor(f"ccin{q}", [128, qb[q], OUT_FT], F32)
            for q in range(NSUB)]
    xfull = [nc.dram_tensor(f"xfull{q}", [subrows[q], OUT_FT], F32,
                            addr_space="Shared") for q in range(NSUB)]

    groups = [list(range(NCORES))]
    gather_q = [0]

    with tile.TileContext(nc) as tc:
        with (
            tc.tile_pool(name="const", bufs=1) as cpool,
            tc.tile_pool(name="psum", bufs=2, space="PSUM") as psum_pool,
        ):
            w_sb = cpool.tile([128, 2, OUT_FT], F32)
            nc.sync.dma_start(out=w_sb[:], in_=w_in[:])
            # bias/128 broadcast, fp16: added into PSUM via a ones matmul
            bias_sb = cpool.tile([128, OUT_FT], F16)
            nc.sync.dma_start(out=bias_sb[:], in_=bias_in[:])
            ones_sb = cpool.tile([128, 128], F16)
            nc.gpsimd.memset(ones_sb[:], 1.0)
            iota_i = cpool.tile([128, 128], I32)
            nc.gpsimd.iota(iota_i[:], pattern=[[1, 128]], base=0,
                           channel_multiplier=0)
            iota_f = cpool.tile([128, 128], F16)
            nc.vector.tensor_copy(out=iota_f[:], in_=iota_i[:])

            # ---- phase 1: x = seq @ W (fp32) + quarter AllGathers ----
            with (
                tc.tile_pool(name="seqpool", bufs=1) as seqpool,
                tc.tile_pool(name="p1work", bufs=3) as p1work,
            ):
                seqT_sb = seqpool.tile([128, 2, NSP], F32)
                # panel loads so early matmuls start promptly
                PAN = 2 * GB * 128          # 2 groups per panel
                for kc in range(2):
                    for p0 in range(0, NSP, PAN):
                        p1 = min(NSP, p0 + PAN)
                        nc.sync.dma_start(out=seqT_sb[:, kc, p0:p1],
                                          in_=seqT[kc, :, p0:p1])

                q = 0
                for g in range(NGROUPS):
                    x_sb = p1work.tile([128, GB, OUT_FT], F32, tag="x_sb")
                    for j in range(GB):
                        nb = g * GB + j
                        px = psum_pool.tile([128, OUT_FT], F32, tag="px")
                        for kc in range(2):
                            nc.tensor.matmul(
                                px[:],
                                seqT_sb[:, kc, nb * 128:(nb + 1) * 128],
                                w_sb[:, kc, :],
                                start=(kc == 0),
                                stop=(kc == 1),
                            )
                        nc.vector.tensor_copy(out=x_sb[:, j, :], in_=px[:])
                    nc.sync.dma_start(
                        out=sf_out[:, g * GB:(g + 1) * GB, :], in_=x_sb[:])
                    gq = g - (int(qg_end[q - 1]) if q else 0)
                    nc.sync.dma_start(
                        out=ccin[q][:, gq * GB:(gq + 1) * GB, :], in_=x_sb[:])
                    if g + 1 == qg_end[q]:
                        nc.gpsimd.collective_compute(
                            "AllGather",
                            mybir.AluOpType.bypass,
                            replica_groups=groups,
                            ins=[ccin[q][:]],
                            outs=[xfull[q][:]],
                        )
                        q += 1

            # ---- phase 2: bulk gather + scale + segment-sum matmul ----
            with tc.tile_pool(name="p2work", bufs=3) as p2:
                for g in range(NGROUPS):
                    gidx_sb = p2.tile([128, gcols], I16, tag="gidx")
                    nc.sync.dma_start(out=gidx_sb[:], in_=gidx[:, g])
                    val_sb = p2.tile([128, sgt], F16, tag="val")
                    nc.sync.dma_start(out=val_sb[:], in_=val[:, g])
                    rl_sb = p2.tile([128, GB, ncht], F16, tag="rl")
                    nc.sync.dma_start(out=rl_sb[:],
                                      in_=rl[:, g * GB:(g + 1) * GB, :])
                    # xg chunk layout per group: bucket-major regions;
                    # bucket b block j chunk cc at reg_b[b] + j*nchb[b] + cc
                    xg = p2.tile([128, sgt, OUT_FT], F32, tag="xg")
                    for b in range(NSUB):
                        sgb = GB * nchb[b]
                        for off in range(0, sgb, MAXCH):
                            ln = min(MAXCH, sgb - off)
                            r0 = int(reg_b[b]) + off
                            c0 = int(gcol_b[b]) + off * 8
                            nc.gpsimd.dma_gather(
                                out_ap=xg[:, r0:r0 + ln, :],
                                in_ap=xfull[b][:],
                                idxs_ap=gidx_sb[:, c0:c0 + ln * 8],
                                num_idxs=ln * 128,
                                num_idxs_reg=ln * 128,
                                elem_size=OUT_FT,
                                queue_num=gather_q[0] % nq,
                            )
                            gather_q[0] += 1
                    # fold edge_val in while casting f32 -> fp16 (one op)
                    xg16 = p2.tile([128, sgt, OUT_FT], F16, tag="xg16")
                    nc.vector.tensor_tensor(
                        out=xg16[:],
                        in0=xg[:],
                        in1=val_sb[:].unsqueeze(2).broadcast_to(
                            [128, sgt, OUT_FT]),
                        op=mybir.AluOpType.mult,
                    )
                    out_sb = p2.tile([128, GB, OUT_FT], F32, tag="out_sb")
                    for j in range(GB):
                        # A[p, c, q] = (rowloc[p, c] == q); -1 pads vanish
                        a_sb = p2.tile([128, ncht * 128], F16, tag="a_sb")
                        nc.vector.tensor_tensor(
                            out=a_sb[:].rearrange("p (c q) -> p c q", q=128),
                            in0=rl_sb[:, j, :].unsqueeze(2).broadcast_to(
                                [128, ncht, 128]),
                            in1=iota_f[:].unsqueeze(1).broadcast_to(
                                [128, ncht, 128]),
                            op=mybir.AluOpType.is_equal,
                        )
                        po = psum_pool.tile([128, OUT_FT], F32, tag="po")
                        nc.tensor.matmul(po[:], ones_sb[:], bias_sb[:],
                                         start=True, stop=False)
                        for b in range(NSUB):
                            for cc in range(nchb[b]):
                                ci = int(off_b[b]) + cc
                                rc = int(reg_b[b]) + j * nchb[b] + cc
                                nc.tensor.matmul(
                                    po[:],
                                    a_sb[:, ci * 128:(ci + 1) * 128],
                                    xg16[:, rc, :],
                                    start=False,
                                    stop=(ci == ncht - 1),
                                )
                        nc.scalar.activation(
                            out=out_sb[:, j, :], in_=po[:],
                            func=mybir.ActivationFunctionType.Relu)
                    nc.sync.dma_start(
                        out=agg_out[:, g * GB:(g + 1) * GB, :], in_=out_sb[:])

    nc.compile()
    return nc


def prepare_inputs(seq, edge_row, edge_col, edge_val, W, b):
    """Host-side sharding / graph partitioning. Returns (in_maps, nchb)."""
    seq = np.asarray(seq, dtype=np.float32).reshape(N_NODES, IN_FT)
    r = np.asarray(edge_row).astype(np.int64)
    c = np.asarray(edge_col).astype(np.int64)
    v = np.asarray(edge_val, dtype=np.float32)
    W = np.asarray(W, dtype=np.float32).reshape(IN_FT, OUT_FT)
    b = np.asarray(b, dtype=np.float32).reshape(OUT_FT)

    qg_end, qb, qb_start, subrows = _quarter_layout()
    qb_start = qb_start.astype(np.int64)
    # quarter (= bucket) of each block index
    blk_q = np.searchsorted(qb_start[1:], np.arange(NB), side="right")

    # feature-table row within its quarter sub-table (partition-major)
    csrc = c // NS
    crem = c % NS
    cblk = crem // 128
    cp = crem % 128
    cq = blk_q[cblk]
    nqb = np.asarray(qb)[cq]
    lidx = (csrc * 128 * nqb + cp * nqb + (cblk - qb_start[cq])).astype(
        np.int16)

    core = r // NS
    loc = r - core * NS
    blk = loc >> 7
    rowloc = (loc & 127).astype(np.float16)
    bucket = cq

    # per-bucket chunk capacity from per-(core, block, bucket) degrees
    key = (core * NB + blk) * NSUB + bucket
    ngrp = NCORES * NB * NSUB
    counts = np.bincount(key, minlength=ngrp).reshape(NCORES, NB, NSUB)
    nchb = [max(1, int(np.ceil(counts[:, :, b].max() / 128)))
            for b in range(NSUB)]
    caps = np.array([c_ * 128 for c_ in nchb])
    off_edge = np.concatenate([[0], np.cumsum(caps)])  # within (core, block)
    tot_cap = int(off_edge[-1])                        # ncht * 128

    order = np.argsort(key, kind="stable")
    key_s = key[order]
    starts = np.searchsorted(key_s, np.arange(ngrp))
    pos = np.arange(N_EDGES) - starts[key_s]
    kb = key_s % NSUB
    kcb = key_s // NSUB          # core * NB + blk
    dest = kcb * tot_cap + off_edge[kb] + pos

    idxp = np.zeros(NCORES * NB * tot_cap, np.int16)       # pad: row 0
    valp = np.zeros(NCORES * NB * tot_cap, np.float16)
    rlp = np.full(NCORES * NB * tot_cap, -1.0, np.float16)  # pad: killed
    idxp[dest] = lidx[order]
    valp[dest] = v[order].astype(np.float16)
    rlp[dest] = rowloc[order]

    # [core, block, chunk(b,cc), lane] views
    idxp = idxp.reshape(NCORES, NB, tot_cap)
    valp = valp.reshape(NCORES, NB, tot_cap)
    rlp = rlp.reshape(NCORES, NB, tot_cap)

    # rl: block-major [core, 128, NB, ncht]
    ncht = sum(nchb)
    rl_l = rlp.reshape(NCORES, NB, ncht, 128).transpose(0, 3, 1, 2)
    rl_arr = np.ascontiguousarray(rl_l)

    # val + gidx: per (group, bucket) streams (blocks of the group concat)
    sgt = GB * ncht
    val_arr = np.empty((NCORES, 128, NGROUPS, sgt), np.float16)
    gcols = sgt * 8
    gidx_arr = np.empty((NCORES, 16, NGROUPS, gcols), np.int16)
    reg0 = 0
    gc0 = 0
    for b_ in range(NSUB):
        cb = caps[b_]
        sgb = GB * nchb[b_]
        e0, e1 = off_edge[b_], off_edge[b_ + 1]
        # [core, group, GB, cb] -> stream [core, group, GB*cb]
        seg_i = idxp[:, :, e0:e1].reshape(NCORES, NGROUPS, GB * cb)
        seg_v = valp[:, :, e0:e1].reshape(NCORES, NGROUPS, GB * cb)
        # val chunk-lane layout [128, chunks]
        vl = seg_v.reshape(NCORES, NGROUPS, sgb, 128).transpose(0, 3, 1, 2)
        val_arr[:, :, :, reg0:reg0 + sgb] = vl
        # idx 16-wrap: i -> [i % 16, i // 16]
        wi = seg_i.reshape(NCORES, NGROUPS, GB * cb // 16, 16)
        gidx_arr[:, :, :, gc0:gc0 + GB * cb // 16] = wi.transpose(0, 3, 1, 2)
        reg0 += sgb
        gc0 += GB * cb // 16
    gidx_full = np.broadcast_to(
        gidx_arr[:, None], (NCORES, 8, 16, NGROUPS, gcols))
    gidx_full = np.ascontiguousarray(
        gidx_full.reshape(NCORES, 128, NGROUPS, gcols))

    biasb = np.broadcast_to((b / 128.0).astype(np.float16),
                            (128, OUT_FT)).copy()
    w3 = np.ascontiguousarray(
        W.reshape(2, 128, OUT_FT).transpose(1, 0, 2))  # [128, 2, OUT_FT]

    in_maps = []
    for k in range(NCORES):
        shard = np.zeros((NSP, IN_FT), np.float32)
        shard[:NS] = seq[k * NS:(k + 1) * NS]
        seqT_k = np.ascontiguousarray(shard.T).reshape(2, 128, NSP)
        in_maps.append({
            "seqT": seqT_k,
            "gidx": gidx_full[k],
            "val": np.ascontiguousarray(val_arr[k]),
            "rl": rl_arr[k],
            "w": w3,
            "biasb": biasb,
        })
    return in_maps, tuple(nchb)


_PROGRAMS: dict[tuple, object] = {}


def kernel(seq, edge_row, edge_col, edge_val, W, b):
    in_maps, nchb = prepare_inputs(seq, edge_row, edge_col, edge_val, W, b)
    prog = _PROGRAMS.get(nchb)
    if prog is None:
        prog = _PROGRAMS[nchb] = build_program(nchb)
    res = run_bass_kernel_spmd(prog, in_maps, core_ids=list(range(NCORES)))

    def unshard(name):
        # [128, NB, OUT_FT] partition-major -> [NS, OUT_FT] row-major
        parts = [
            res.results[k][name].transpose(1, 0, 2).reshape(NSP, OUT_FT)[:NS]
            for k in range(NCORES)
        ]
        return np.concatenate(parts)[None]

    return unshard("agg"), unshard("sf")



# revision 17
# speedup vs baseline: 1.1581x; 1.1581x over previous
"""GCN layer (dense projection + sparse neighbor aggregation) on 8 Trainium2
NeuronCores via Bass/Tile.

Strategy: shard nodes (and their incident edges, grouped by destination row)
across the 8 cores; replicate W/b; AllGather the projected node features so
every core can gather arbitrary source columns (split into 4 bucket-aligned
sub-collectives pipelined with the projection); per 128-row output block,
bulk-gather the needed source rows with DMAGatherAnt (int16 indices into 4
sub-tables of <=32k rows, ONE gather instruction per (group, bucket) to
amortize the ~1us SWDGE fixed cost), scale by edge_val, and segment-sum via
an assignment-matrix matmul accumulated in PSUM (bias folded in as an extra
rank-128 matmul; padded gather slots are killed by rowloc=-1).

Chunk capacities are per-(block, bucket) (max over the 8 cores only), cutting
gather padding vs. a single global capacity.
"""

import sys

if "/opt/trn_rl_repo" not in sys.path:
    sys.path.insert(0, "/opt/trn_rl_repo")

import numpy as np

import concourse.bass as bass
import concourse.mybir as mybir
import concourse.tile as tile
from concourse import bacc
from concourse.bass_utils import run_bass_kernel_spmd

N_NODES = 100000
N_EDGES = 1600000
IN_FT = 256
OUT_FT = 64
NCORES = 8
NS = N_NODES // NCORES          # 12500 nodes per core
NB = (NS + 127) // 128          # 98 row blocks per core
NSP = NB * 128                  # 12544 padded nodes per core
GB = 7                          # row blocks per pipeline group (98 = 14 * 7)
NGROUPS = NB // GB              # 14
QGROUPS = [2, 4, 4, 4]          # groups per sub-collective (sums to 14)
NSUB = len(QGROUPS)

F32 = mybir.dt.float32
F16 = mybir.dt.float16
BF16 = mybir.dt.bfloat16
I32 = mybir.dt.int32
I16 = mybir.dt.int16

NQ = 4                          # SWDGE queues
MAXI = 1024                     # max indices per dma_gather (HW/ucode cap)


def _quarter_layout():
    qg_end = np.cumsum(QGROUPS)                  # groups per bucket, cumul
    qb = [g * GB for g in QGROUPS]               # blocks per bucket
    qb_start = np.concatenate([[0], np.cumsum(qb)])
    subrows = [NCORES * 128 * n for n in qb]
    return qg_end, qb, qb_start, subrows


class Layout:
    """Derived index layout shared by host prep and program build.

    caps[j][b]: 128-edge chunk capacity of (dest block j, source bucket b).
    """

    def __init__(self, caps):
        self.caps = caps = np.asarray(caps)          # [NB, NSUB]
        qg_end, qb, qb_start, subrows = _quarter_layout()
        self.qg_end, self.qb, self.qb_start, self.subrows = (
            qg_end, qb, qb_start, subrows)
        self.ncht = caps.sum(axis=1)                 # chunks per block
        self.maxncht = int(self.ncht.max())
        # per-group stream: bucket-major regions, block-major within bucket
        self.sgb = np.zeros((NGROUPS, NSUB), np.int64)
        for g in range(NGROUPS):
            for b in range(NSUB):
                self.sgb[g, b] = caps[g * GB:(g + 1) * GB, b].sum()
        self.sgt = self.sgb.sum(axis=1)              # chunks per group
        # region offset of bucket b within group g's stream
        self.reg = np.zeros((NGROUPS, NSUB + 1), np.int64)
        self.reg[:, 1:] = np.cumsum(self.sgb, axis=1)
        # offset of block j's chunks within (group, bucket) region
        self.blkoff = np.zeros((NGROUPS, NSUB, GB + 1), np.int64)
        for g in range(NGROUPS):
            for b in range(NSUB):
                self.blkoff[g, b, 1:] = np.cumsum(
                    caps[g * GB:(g + 1) * GB, b])
        # flat DRAM offsets
        self.g_off = np.concatenate([[0], np.cumsum(self.sgt)])  # val cols
        self.total_sgt = int(self.g_off[-1])
        self.rl_off = np.concatenate([[0], np.cumsum(self.ncht)])  # rl cols
        self.total_ncht = int(self.rl_off[-1])

    def key(self):
        return tuple(map(int, self.caps.reshape(-1)))


def build_program(lay: Layout):
    """One SPMD Bass program; all 8 cores run it on their own shards."""
    caps, reg, blkoff, sgb = lay.caps, lay.reg, lay.blkoff, lay.sgb
    qg_end, qb, subrows = lay.qg_end, lay.qb, lay.subrows

    nc = bacc.Bacc("TRN2", target_bir_lowering=False, debug=False,
                   num_devices=NCORES, num_swdge_queues=NQ)

    seqT = nc.dram_tensor("seqT", [2, 128, NSP], BF16, kind="ExternalInput")
    gidx = nc.dram_tensor("gidx", [128, lay.total_sgt * 8], I16,
                          kind="ExternalInput")
    val = nc.dram_tensor("val", [128, lay.total_sgt], F16,
                         kind="ExternalInput")
    rl = nc.dram_tensor("rl", [128, lay.total_ncht], F16,
                        kind="ExternalInput")
    w_in = nc.dram_tensor("w", [128, 2, OUT_FT], BF16, kind="ExternalInput")
    bias_in = nc.dram_tensor("biasb", [128, OUT_FT], F16,
                             kind="ExternalInput")
    # partition-major layouts: [p, block, feature]; host un-permutes
    sf_out = nc.dram_tensor("sf", [128, NB, OUT_FT], F32,
                            kind="ExternalOutput")
    agg_out = nc.dram_tensor("agg", [128, NB, OUT_FT], F32,
                             kind="ExternalOutput")
    ccin = [nc.dram_tensor(f"ccin{b}", [128, qb[b], OUT_FT], F32)
            for b in range(NSUB)]
    xfull = [nc.dram_tensor(f"xfull{b}", [subrows[b], OUT_FT], F32,
                            addr_space="Shared") for b in range(NSUB)]

    groups = [list(range(NCORES))]

    with tile.TileContext(nc) as tc:
        with (
            tc.tile_pool(name="const", bufs=1) as cpool,
            tc.tile_pool(name="psum", bufs=2, space="PSUM") as psum_pool,
        ):
            w_sb = cpool.tile([128, 2, OUT_FT], BF16)
            nc.sync.dma_start(out=w_sb[:], in_=w_in[:])
            # bias/128 broadcast, fp16: added into PSUM via a ones matmul
            bias_sb = cpool.tile([128, OUT_FT], F16)
            nc.sync.dma_start(out=bias_sb[:], in_=bias_in[:])
            ones_sb = cpool.tile([128, 128], F16)
            nc.gpsimd.memset(ones_sb[:], 1.0)
            # iota-tiled constant: col (c*128 + q) = q, f16
            iota_i = cpool.tile([128, lay.maxncht, 128], I32)
            nc.gpsimd.iota(iota_i[:], pattern=[[0, lay.maxncht], [1, 128]],
                           base=0, channel_multiplier=0)
            iota_f = cpool.tile([128, lay.maxncht, 128], F16)
            nc.vector.tensor_copy(out=iota_f[:], in_=iota_i[:])

            # ---- phase 1: x = seq @ W (fp32) + bucket AllGathers ----
            with (
                tc.tile_pool(name="seqpool", bufs=1) as seqpool,
                tc.tile_pool(name="p1work", bufs=3) as p1work,
            ):
                seqT_sb = seqpool.tile([128, 2, NSP], BF16)
                # panel loads so early matmuls start promptly
                PAN = 2 * GB * 128          # 2 groups per panel
                for kc in range(2):
                    for p0 in range(0, NSP, PAN):
                        p1 = min(NSP, p0 + PAN)
                        nc.sync.dma_start(out=seqT_sb[:, kc, p0:p1],
                                          in_=seqT[kc, :, p0:p1])

                q = 0
                for g in range(NGROUPS):
                    x_sb = p1work.tile([128, GB, OUT_FT], F32, tag="x_sb")
                    for j in range(GB):
                        nb = g * GB + j
                        px = psum_pool.tile([128, OUT_FT], F32, tag="px")
                        for kc in range(2):
                            nc.tensor.matmul(
                                px[:],
                                seqT_sb[:, kc, nb * 128:(nb + 1) * 128],
                                w_sb[:, kc, :],
                                start=(kc == 0),
                                stop=(kc == 1),
                            )
                        nc.vector.tensor_copy(out=x_sb[:, j, :], in_=px[:])
                    nc.sync.dma_start(
                        out=sf_out[:, g * GB:(g + 1) * GB, :], in_=x_sb[:])
                    gq = g - (int(qg_end[q - 1]) if q else 0)
                    nc.sync.dma_start(
                        out=ccin[q][:, gq * GB:(gq + 1) * GB, :], in_=x_sb[:])
                    if g + 1 == qg_end[q]:
                        nc.gpsimd.collective_compute(
                            "AllGather",
                            mybir.AluOpType.bypass,
                            replica_groups=groups,
                            ins=[ccin[q][:]],
                            outs=[xfull[q][:]],
                        )
                        q += 1

            # ---- phase 2: bulk gather + scale + segment-sum matmul ----
            gq_ctr = [0]
            with (
                tc.tile_pool(name="p2work", bufs=2) as p2,
                tc.tile_pool(name="p2xg", bufs=3) as p2xg,
            ):
                for g in range(NGROUPS):
                    sgt_g = int(lay.sgt[g])
                    gidx_sb = p2.tile([128, sgt_g * 8], I16, tag="gidx")
                    nc.sync.dma_start(
                        out=gidx_sb[:],
                        in_=gidx[:, lay.g_off[g] * 8:lay.g_off[g + 1] * 8])
                    val_sb = p2.tile([128, sgt_g], F16, tag="val")
                    nc.sync.dma_start(
                        out=val_sb[:],
                        in_=val[:, lay.g_off[g]:lay.g_off[g + 1]])
                    rlc0 = int(lay.rl_off[g * GB])
                    rlc1 = int(lay.rl_off[(g + 1) * GB])
                    rl_sb = p2.tile([128, rlc1 - rlc0], F16, tag="rl")
                    nc.sync.dma_start(out=rl_sb[:], in_=rl[:, rlc0:rlc1])
                    # xg chunk layout per group: bucket-major regions,
                    # block-major within bucket
                    xg = p2xg.tile([128, sgt_g, OUT_FT], F32, tag="xg")
                    for b in range(NSUB):
                        nch = int(sgb[g, b])
                        if nch == 0:
                            continue
                        # split so one instruction's descriptors fit the ring
                        npc = -(-nch * 128 // MAXI)       # pieces
                        per = -(-nch // npc)              # chunks per piece
                        for off in range(0, nch, per):
                            ln = min(per, nch - off)
                            r0 = int(reg[g, b]) + off
                            nc.gpsimd.dma_gather(
                                out_ap=xg[:, r0:r0 + ln, :],
                                in_ap=xfull[b][:],
                                idxs_ap=gidx_sb[:, r0 * 8:(r0 + ln) * 8],
                                num_idxs=ln * 128,
                                num_idxs_reg=ln * 128,
                                elem_size=OUT_FT,
                                queue_num=gq_ctr[0] % NQ,
                            )
                            gq_ctr[0] += 1
                    # fold edge_val in while casting f32 -> fp16 (one op)
                    xg16 = p2.tile([128, sgt_g, OUT_FT], F16, tag="xg16")
                    nc.vector.tensor_tensor(
                        out=xg16[:],
                        in0=xg[:],
                        in1=val_sb[:].unsqueeze(2).broadcast_to(
                            [128, sgt_g, OUT_FT]),
                        op=mybir.AluOpType.mult,
                    )
                    out_sb = p2.tile([128, GB, OUT_FT], F32, tag="out_sb")
                    for j in range(GB):
                        jg = g * GB + j
                        ncht_j = int(lay.ncht[jg])
                        # A[p, c, q] = (rowloc[p, c] == q); -1 pads vanish
                        a_sb = p2.tile([128, lay.maxncht * 128], F16,
                                       tag="a_sb")
                        c0 = int(lay.rl_off[jg]) - rlc0
                        nc.vector.tensor_tensor(
                            out=a_sb[:, :ncht_j * 128].rearrange(
                                "p (c q) -> p c q", q=128),
                            in0=rl_sb[:, c0:c0 + ncht_j].unsqueeze(
                                2).broadcast_to([128, ncht_j, 128]),
                            in1=iota_f[:, :ncht_j, :],
                            op=mybir.AluOpType.is_equal,
                        )
                        po = psum_pool.tile([128, OUT_FT], F32, tag="po")
                        nc.tensor.matmul(po[:], ones_sb[:], bias_sb[:],
                                         start=True, stop=False)
                        ci = 0
                        for b in range(NSUB):
                            nchjb = int(caps[jg, b])
                            rcb = int(reg[g, b] + blkoff[g, b, j])
                            for cc in range(nchjb):
                                nc.tensor.matmul(
                                    po[:],
                                    a_sb[:, (ci + cc) * 128:
                                         (ci + cc + 1) * 128],
                                    xg16[:, rcb + cc, :],
                                    start=False,
                                    stop=(ci + cc == ncht_j - 1),
                                )
                            ci += nchjb
                        nc.scalar.activation(
                            out=out_sb[:, j, :], in_=po[:],
                            func=mybir.ActivationFunctionType.Relu)
                    nc.sync.dma_start(
                        out=agg_out[:, g * GB:(g + 1) * GB, :], in_=out_sb[:])

    nc.compile()
    return nc


def prepare_inputs(seq, edge_row, edge_col, edge_val, W, b):
    """Host-side sharding / graph partitioning. Returns (in_maps, layout)."""
    seq = np.asarray(seq, dtype=np.float32).reshape(N_NODES, IN_FT)
    r = np.asarray(edge_row).astype(np.int64)
    c = np.asarray(edge_col).astype(np.int64)
    v = np.asarray(edge_val, dtype=np.float32)
    W = np.asarray(W, dtype=np.float32).reshape(IN_FT, OUT_FT)
    b = np.asarray(b, dtype=np.float32).reshape(OUT_FT)

    qg_end, qb, qb_start, subrows = _quarter_layout()
    qb_start = qb_start.astype(np.int64)
    # bucket of each block index
    blk_q = np.searchsorted(qb_start[1:], np.arange(NB), side="right")

    # feature-table row within its bucket sub-table (core, partition, block)
    csrc = c // NS
    crem = c % NS
    cblk = crem // 128
    cp = crem % 128
    cq = blk_q[cblk]
    nqb = np.asarray(qb)[cq]
    lidx = (csrc * 128 * nqb + cp * nqb + (cblk - qb_start[cq])).astype(
        np.int16)

    core = r // NS
    loc = r - core * NS
    blk = loc >> 7
    rowloc = (loc & 127).astype(np.float16)
    bucket = cq

    # per-(block, bucket) chunk capacity: max count over the 8 cores
    key = (core * NB + blk) * NSUB + bucket
    ngrp = NCORES * NB * NSUB
    counts = np.bincount(key, minlength=ngrp).reshape(NCORES, NB, NSUB)
    caps = np.ceil(counts.max(axis=0) / 128).astype(np.int64)  # [NB, NSUB]
    caps = np.maximum(caps, 1)
    lay = Layout(caps)

    # edge destination slot within the flat per-core stream; edges within a
    # (core, block, bucket) run sorted by source row for HBM read locality
    order = np.lexsort((lidx, key))
    key_s = key[order]
    starts = np.searchsorted(key_s, np.arange(ngrp))
    pos = np.arange(N_EDGES) - starts[key_s]           # rank within (c,j,b)
    kb = key_s % NSUB
    kj = (key_s // NSUB) % NB
    kcore = key_s // (NSUB * NB)
    kg = kj // GB
    kjl = kj % GB
    # chunk column within the group stream; lane within chunk
    sc = (lay.reg[kg, kb] + lay.blkoff[kg, kb, kjl] + pos // 128)
    lane = pos % 128
    # global chunk col across groups (flat val layout)
    gchunk = lay.g_off[kg] + sc
    # rl col within flat rl layout
    capcum = np.zeros((NB, NSUB + 1), np.int64)
    capcum[:, 1:] = np.cumsum(caps, axis=1)
    rlcol = lay.rl_off[kj] + capcum[kj, kb] + pos // 128

    tot_chunks = lay.total_sgt
    valp = np.zeros((NCORES, 128, tot_chunks), np.float16)
    idxp = np.zeros((NCORES, 128, tot_chunks), np.int16)   # pad: row 0
    rlp = np.full((NCORES, 128, lay.total_ncht), -1.0, np.float16)
    valp[kcore, lane, gchunk] = v[order].astype(np.float16)
    idxp[kcore, lane, gchunk] = lidx[order]
    rlp[kcore, lane, rlcol] = rowloc[order]

    # gidx 16-wrap per group: idx stream i -> [i % 16, i // 16], x8 replicate
    gidx_arr = np.empty((NCORES, 16, tot_chunks * 8), np.int16)
    for g in range(NGROUPS):
        s0, s1 = int(lay.g_off[g]), int(lay.g_off[g + 1])
        seg = idxp[:, :, s0:s1]                      # [NCORES, 128lane, sg]
        # stream order: chunk-major, lane-minor -> i = sc*128 + lane
        stream = seg.transpose(0, 2, 1).reshape(NCORES, (s1 - s0) * 128)
        wi = stream.reshape(NCORES, (s1 - s0) * 8, 16)
        gidx_arr[:, :, s0 * 8:s1 * 8] = wi.transpose(0, 2, 1)
    gidx_full = np.broadcast_to(
        gidx_arr[:, None], (NCORES, 8, 16, tot_chunks * 8))
    gidx_full = np.ascontiguousarray(
        gidx_full.reshape(NCORES, 128, tot_chunks * 8))

    import ml_dtypes

    biasb = np.broadcast_to((b / 128.0).astype(np.float16),
                            (128, OUT_FT)).copy()
    w3 = np.ascontiguousarray(
        W.reshape(2, 128, OUT_FT).transpose(1, 0, 2)).astype(
            ml_dtypes.bfloat16)  # [128, 2, OUT_FT]

    in_maps = []
    for k in range(NCORES):
        shard = np.zeros((NSP, IN_FT), np.float32)
        shard[:NS] = seq[k * NS:(k + 1) * NS]
        seqT_k = np.ascontiguousarray(shard.T).reshape(2, 128, NSP).astype(
            ml_dtypes.bfloat16)
        in_maps.append({
            "seqT": seqT_k,
            "gidx": gidx_full[k],
            "val": np.ascontiguousarray(valp[k]),
            "rl": np.ascontiguousarray(rlp[k]),
            "w": w3,
            "biasb": biasb,
        })
    return in_maps, lay


_PROGRAMS: dict[tuple, object] = {}


def kernel(seq, edge_row, edge_col, edge_val, W, b):
    in_maps, lay = prepare_inputs(seq, edge_row, edge_col, edge_val, W, b)
    prog = _PROGRAMS.get(lay.key())
    if prog is None:
        prog = _PROGRAMS[lay.key()] = build_program(lay)
    res = run_bass_kernel_spmd(prog, in_maps, core_ids=list(range(NCORES)))

    def unshard(name):
        # [128, NB, OUT_FT] partition-major -> [NS, OUT_FT] row-major
        parts = [
            res.results[k][name].transpose(1, 0, 2).reshape(NSP, OUT_FT)[:NS]
            for k in range(NCORES)
        ]
        return np.concatenate(parts)[None]

    return unshard("agg"), unshard("sf")


# revision 20
# speedup vs baseline: 1.2425x; 1.0729x over previous
"""GCN layer (dense projection + sparse neighbor aggregation) on 8 Trainium2
NeuronCores via Bass/Tile.

Strategy: shard nodes (and their incident edges, grouped by destination row)
across the 8 cores; replicate W/b; AllGather the projected node features so
every core can gather arbitrary source columns (split into 4 bucket-aligned
sub-collectives pipelined with the projection); per 128-row output block,
bulk-gather the needed source rows with DMAGatherAnt (int16 indices into 4
sub-tables of <=32k rows, ONE gather instruction per (group, bucket) to
amortize the ~1us SWDGE fixed cost), scale by edge_val, and segment-sum via
an assignment-matrix matmul accumulated in PSUM (bias folded in as an extra
rank-128 matmul; padded gather slots are killed by rowloc=-1).

Chunk capacities are per-(block, bucket) (max over the 8 cores only), cutting
gather padding vs. a single global capacity.
"""

import sys

if "/opt/trn_rl_repo" not in sys.path:
    sys.path.insert(0, "/opt/trn_rl_repo")

import numpy as np

import concourse.bass as bass
import concourse.mybir as mybir
import concourse.tile as tile
from concourse import bacc
from concourse.bass_utils import run_bass_kernel_spmd

N_NODES = 100000
N_EDGES = 1600000
IN_FT = 256
OUT_FT = 64
NCORES = 8
NS = N_NODES // NCORES          # 12500 nodes per core
NB = (NS + 127) // 128          # 98 row blocks per core
NSP = NB * 128                  # 12544 padded nodes per core
GB = 7                          # row blocks per pipeline group (98 = 14 * 7)
NGROUPS = NB // GB              # 14
QGROUPS = [2, 4, 4, 4]          # groups per sub-collective (sums to 14)
NSUB = len(QGROUPS)

F32 = mybir.dt.float32
F16 = mybir.dt.float16
BF16 = mybir.dt.bfloat16
I32 = mybir.dt.int32
I16 = mybir.dt.int16

NQ = 4                          # SWDGE queues
MAXI = 1024                     # max indices per dma_gather (HW/ucode cap)


def _quarter_layout():
    qg_end = np.cumsum(QGROUPS)                  # groups per bucket, cumul
    qb = [g * GB for g in QGROUPS]               # blocks per bucket
    qb_start = np.concatenate([[0], np.cumsum(qb)])
    subrows = [NCORES * 128 * n for n in qb]
    return qg_end, qb, qb_start, subrows


class Layout:
    """Derived index layout shared by host prep and program build.

    caps[j][b]: 128-edge chunk capacity of (dest block j, source bucket b).
    """

    def __init__(self, caps):
        self.caps = caps = np.asarray(caps)          # [NB, NSUB]
        qg_end, qb, qb_start, subrows = _quarter_layout()
        self.qg_end, self.qb, self.qb_start, self.subrows = (
            qg_end, qb, qb_start, subrows)
        self.ncht = caps.sum(axis=1)                 # chunks per block
        self.maxncht = int(self.ncht.max())
        # per-group stream: bucket-major regions, block-major within bucket
        self.sgb = np.zeros((NGROUPS, NSUB), np.int64)
        for g in range(NGROUPS):
            for b in range(NSUB):
                self.sgb[g, b] = caps[g * GB:(g + 1) * GB, b].sum()
        self.sgt = self.sgb.sum(axis=1)              # chunks per group
        # region offset of bucket b within group g's stream
        self.reg = np.zeros((NGROUPS, NSUB + 1), np.int64)
        self.reg[:, 1:] = np.cumsum(self.sgb, axis=1)
        # offset of block j's chunks within (group, bucket) region
        self.blkoff = np.zeros((NGROUPS, NSUB, GB + 1), np.int64)
        for g in range(NGROUPS):
            for b in range(NSUB):
                self.blkoff[g, b, 1:] = np.cumsum(
                    caps[g * GB:(g + 1) * GB, b])
        # flat DRAM offsets
        self.g_off = np.concatenate([[0], np.cumsum(self.sgt)])  # val cols
        self.total_sgt = int(self.g_off[-1])
        self.rl_off = np.concatenate([[0], np.cumsum(self.ncht)])  # rl cols
        self.total_ncht = int(self.rl_off[-1])

    def key(self):
        return tuple(map(int, self.caps.reshape(-1)))


def build_program(lay: Layout):
    """One SPMD Bass program; all 8 cores run it on their own shards."""
    caps, reg, blkoff, sgb = lay.caps, lay.reg, lay.blkoff, lay.sgb
    qg_end, qb, subrows = lay.qg_end, lay.qb, lay.subrows

    nc = bacc.Bacc("TRN2", target_bir_lowering=False, debug=False,
                   num_devices=NCORES, num_swdge_queues=NQ)

    seqT = nc.dram_tensor("seqT", [2, 128, NSP], BF16, kind="ExternalInput")
    gidx = nc.dram_tensor("gidx", [128, lay.total_sgt * 8], I16,
                          kind="ExternalInput")
    val = nc.dram_tensor("val", [128, lay.total_sgt], F16,
                         kind="ExternalInput")
    rl = nc.dram_tensor("rl", [128, lay.total_ncht], F16,
                        kind="ExternalInput")
    w_in = nc.dram_tensor("w", [128, 2, OUT_FT], BF16, kind="ExternalInput")
    bias_in = nc.dram_tensor("biasb", [128, OUT_FT], F16,
                             kind="ExternalInput")
    # partition-major layouts: [p, block, feature]; host un-permutes
    sf_out = nc.dram_tensor("sf", [128, NB, OUT_FT], F32,
                            kind="ExternalOutput")
    agg_out = nc.dram_tensor("agg", [128, NB, OUT_FT], F32,
                             kind="ExternalOutput")
    ccin = [nc.dram_tensor(f"ccin{b}", [128, qb[b], OUT_FT], F32)
            for b in range(NSUB)]
    xfull = [nc.dram_tensor(f"xfull{b}", [subrows[b], OUT_FT], F32,
                            addr_space="Shared") for b in range(NSUB)]

    groups = [list(range(NCORES))]

    with tile.TileContext(nc) as tc:
        with (
            tc.tile_pool(name="const", bufs=1) as cpool,
            tc.tile_pool(name="psum", bufs=2, space="PSUM") as psum_pool,
        ):
            w_sb = cpool.tile([128, 2, OUT_FT], BF16)
            nc.sync.dma_start(out=w_sb[:], in_=w_in[:])
            # bias/128 broadcast, fp16: added into PSUM via a ones matmul
            bias_sb = cpool.tile([128, OUT_FT], F16)
            nc.sync.dma_start(out=bias_sb[:], in_=bias_in[:])
            ones_sb = cpool.tile([128, 128], F16)
            nc.gpsimd.memset(ones_sb[:], 1.0)
            # iota-tiled constant: col (c*128 + q) = q, f16
            iota_i = cpool.tile([128, lay.maxncht, 128], I32)
            nc.gpsimd.iota(iota_i[:], pattern=[[0, lay.maxncht], [1, 128]],
                           base=0, channel_multiplier=0)
            iota_f = cpool.tile([128, lay.maxncht, 128], F16)
            nc.vector.tensor_copy(out=iota_f[:], in_=iota_i[:])

            # all index/val/rowloc data loaded before phase 1 queues the
            # projection-dependent writes (sync engine is in-order), so the
            # loads land during the seq-load window
            gidx_sb = cpool.tile([128, lay.total_sgt * 8], I16)
            nc.sync.dma_start(out=gidx_sb[:], in_=gidx[:])
            val_sb = cpool.tile([128, lay.total_sgt], F16)
            nc.sync.dma_start(out=val_sb[:], in_=val[:])
            rl_sb = cpool.tile([128, lay.total_ncht], F16)
            nc.sync.dma_start(out=rl_sb[:], in_=rl[:])

            # ---- phase 1: x = seq @ W (fp32) + bucket AllGathers ----
            with (
                tc.tile_pool(name="seqpool", bufs=1) as seqpool,
                tc.tile_pool(name="p1work", bufs=3) as p1work,
            ):
                seqT_sb = seqpool.tile([128, 2, NSP], BF16)
                # panel loads so early matmuls start promptly
                PAN = 2 * GB * 128          # 2 groups per panel
                for kc in range(2):
                    for p0 in range(0, NSP, PAN):
                        p1 = min(NSP, p0 + PAN)
                        nc.sync.dma_start(out=seqT_sb[:, kc, p0:p1],
                                          in_=seqT[kc, :, p0:p1])

                q = 0
                for g in range(NGROUPS):
                    x_sb = p1work.tile([128, GB, OUT_FT], F32, tag="x_sb")
                    for j in range(GB):
                        nb = g * GB + j
                        px = psum_pool.tile([128, OUT_FT], F32, tag="px")
                        for kc in range(2):
                            nc.tensor.matmul(
                                px[:],
                                seqT_sb[:, kc, nb * 128:(nb + 1) * 128],
                                w_sb[:, kc, :],
                                start=(kc == 0),
                                stop=(kc == 1),
                            )
                        nc.vector.tensor_copy(out=x_sb[:, j, :], in_=px[:])
                    nc.sync.dma_start(
                        out=sf_out[:, g * GB:(g + 1) * GB, :], in_=x_sb[:])
                    gq = g - (int(qg_end[q - 1]) if q else 0)
                    nc.sync.dma_start(
                        out=ccin[q][:, gq * GB:(gq + 1) * GB, :], in_=x_sb[:])
                    if g + 1 == qg_end[q]:
                        nc.gpsimd.collective_compute(
                            "AllGather",
                            mybir.AluOpType.bypass,
                            replica_groups=groups,
                            ins=[ccin[q][:]],
                            outs=[xfull[q][:]],
                        )
                        q += 1

            # ---- phase 2: bulk gather + scale + segment-sum matmul ----
            gq_ctr = [0]
            with (
                tc.tile_pool(name="p2work", bufs=2) as p2,
                tc.tile_pool(name="p2xg", bufs=2) as p2xg,
            ):
                for g in range(NGROUPS):
                    sgt_g = int(lay.sgt[g])
                    go = int(lay.g_off[g])
                    # xg chunk layout per group: bucket-major regions,
                    # block-major within bucket
                    xg = p2xg.tile([128, sgt_g, OUT_FT], F32, tag="xg")
                    xg16 = p2.tile([128, sgt_g, OUT_FT], F16, tag="xg16")
                    for b in range(NSUB):
                        nch = int(sgb[g, b])
                        if nch == 0:
                            continue
                        # split so one instruction's descriptors fit the ring
                        npc = -(-nch * 128 // MAXI)       # pieces
                        per = -(-nch // npc)              # chunks per piece
                        for off in range(0, nch, per):
                            ln = min(per, nch - off)
                            r0 = int(reg[g, b]) + off
                            nc.gpsimd.dma_gather(
                                out_ap=xg[:, r0:r0 + ln, :],
                                in_ap=xfull[b][:],
                                idxs_ap=gidx_sb[:, (go + r0) * 8:
                                                (go + r0 + ln) * 8],
                                num_idxs=ln * 128,
                                num_idxs_reg=ln * 128,
                                elem_size=OUT_FT,
                                queue_num=gq_ctr[0] % NQ,
                            )
                            gq_ctr[0] += 1
                        # fold edge_val in while casting f32 -> fp16, per
                        # bucket region so xg frees early and the last
                        # group's tail is short
                        r0 = int(reg[g, b])
                        nc.vector.tensor_tensor(
                            out=xg16[:, r0:r0 + nch, :],
                            in0=xg[:, r0:r0 + nch, :],
                            in1=val_sb[:, go + r0:go + r0 + nch].unsqueeze(
                                2).broadcast_to([128, nch, OUT_FT]),
                            op=mybir.AluOpType.mult,
                        )
                    out_sb = p2.tile([128, GB, OUT_FT], F32, tag="out_sb")
                    for j in range(GB):
                        jg = g * GB + j
                        ncht_j = int(lay.ncht[jg])
                        # A[p, c, q] = (rowloc[p, c] == q); -1 pads vanish
                        a_sb = p2.tile([128, lay.maxncht * 128], F16,
                                       tag="a_sb")
                        c0 = int(lay.rl_off[jg])
                        nc.vector.tensor_tensor(
                            out=a_sb[:, :ncht_j * 128].rearrange(
                                "p (c q) -> p c q", q=128),
                            in0=iota_f[:, :ncht_j, :],
                            in1=rl_sb[:, c0:c0 + ncht_j].unsqueeze(
                                2).broadcast_to([128, ncht_j, 128]),
                            op=mybir.AluOpType.is_equal,
                        )
                        po = psum_pool.tile([128, OUT_FT], F32, tag="po")
                        nc.tensor.matmul(po[:], ones_sb[:], bias_sb[:],
                                         start=True, stop=False)
                        ci = 0
                        for b in range(NSUB):
                            nchjb = int(caps[jg, b])
                            rcb = int(reg[g, b] + blkoff[g, b, j])
                            for cc in range(nchjb):
                                nc.tensor.matmul(
                                    po[:],
                                    a_sb[:, (ci + cc) * 128:
                                         (ci + cc + 1) * 128],
                                    xg16[:, rcb + cc, :],
                                    start=False,
                                    stop=(ci + cc == ncht_j - 1),
                                )
                            ci += nchjb
                        nc.scalar.activation(
                            out=out_sb[:, j, :], in_=po[:],
                            func=mybir.ActivationFunctionType.Relu)
                    nc.sync.dma_start(
                        out=agg_out[:, g * GB:(g + 1) * GB, :], in_=out_sb[:])

    nc.compile()
    return nc


def prepare_inputs(seq, edge_row, edge_col, edge_val, W, b):
    """Host-side sharding / graph partitioning. Returns (in_maps, layout)."""
    seq = np.asarray(seq, dtype=np.float32).reshape(N_NODES, IN_FT)
    r = np.asarray(edge_row).astype(np.int64)
    c = np.asarray(edge_col).astype(np.int64)
    v = np.asarray(edge_val, dtype=np.float32)
    W = np.asarray(W, dtype=np.float32).reshape(IN_FT, OUT_FT)
    b = np.asarray(b, dtype=np.float32).reshape(OUT_FT)

    qg_end, qb, qb_start, subrows = _quarter_layout()
    qb_start = qb_start.astype(np.int64)
    # bucket of each block index
    blk_q = np.searchsorted(qb_start[1:], np.arange(NB), side="right")

    # feature-table row within its bucket sub-table (core, partition, block)
    csrc = c // NS
    crem = c % NS
    cblk = crem // 128
    cp = crem % 128
    cq = blk_q[cblk]
    nqb = np.asarray(qb)[cq]
    lidx = (csrc * 128 * nqb + cp * nqb + (cblk - qb_start[cq])).astype(
        np.int16)

    core = r // NS
    loc = r - core * NS
    blk = loc >> 7
    rowloc = (loc & 127).astype(np.float16)
    bucket = cq

    # per-(block, bucket) chunk capacity: max count over the 8 cores
    key = (core * NB + blk) * NSUB + bucket
    ngrp = NCORES * NB * NSUB
    counts = np.bincount(key, minlength=ngrp).reshape(NCORES, NB, NSUB)
    caps = np.ceil(counts.max(axis=0) / 128).astype(np.int64)  # [NB, NSUB]
    caps = np.maximum(caps, 1)
    lay = Layout(caps)

    # edge destination slot within the flat per-core stream; edges within a
    # (core, block, bucket) run sorted by source row for HBM read locality
    order = np.lexsort((lidx, key))
    key_s = key[order]
    starts = np.searchsorted(key_s, np.arange(ngrp))
    pos = np.arange(N_EDGES) - starts[key_s]           # rank within (c,j,b)
    kb = key_s % NSUB
    kj = (key_s // NSUB) % NB
    kcore = key_s // (NSUB * NB)
    kg = kj // GB
    kjl = kj % GB
    # chunk column within the group stream; lane within chunk
    sc = (lay.reg[kg, kb] + lay.blkoff[kg, kb, kjl] + pos // 128)
    lane = pos % 128
    # global chunk col across groups (flat val layout)
    gchunk = lay.g_off[kg] + sc
    # rl col within flat rl layout
    capcum = np.zeros((NB, NSUB + 1), np.int64)
    capcum[:, 1:] = np.cumsum(caps, axis=1)
    rlcol = lay.rl_off[kj] + capcum[kj, kb] + pos // 128

    tot_chunks = lay.total_sgt
    valp = np.zeros((NCORES, 128, tot_chunks), np.float16)
    idxp = np.zeros((NCORES, 128, tot_chunks), np.int16)   # pad: row 0
    rlp = np.full((NCORES, 128, lay.total_ncht), -1.0, np.float16)
    valp[kcore, lane, gchunk] = v[order].astype(np.float16)
    idxp[kcore, lane, gchunk] = lidx[order]
    rlp[kcore, lane, rlcol] = rowloc[order]

    # gidx 16-wrap per group: idx stream i -> [i % 16, i // 16], x8 replicate
    gidx_arr = np.empty((NCORES, 16, tot_chunks * 8), np.int16)
    for g in range(NGROUPS):
        s0, s1 = int(lay.g_off[g]), int(lay.g_off[g + 1])
        seg = idxp[:, :, s0:s1]                      # [NCORES, 128lane, sg]
        # stream order: chunk-major, lane-minor -> i = sc*128 + lane
        stream = seg.transpose(0, 2, 1).reshape(NCORES, (s1 - s0) * 128)
        wi = stream.reshape(NCORES, (s1 - s0) * 8, 16)
        gidx_arr[:, :, s0 * 8:s1 * 8] = wi.transpose(0, 2, 1)
    gidx_full = np.broadcast_to(
        gidx_arr[:, None], (NCORES, 8, 16, tot_chunks * 8))
    gidx_full = np.ascontiguousarray(
        gidx_full.reshape(NCORES, 128, tot_chunks * 8))

    import ml_dtypes

    biasb = np.broadcast_to((b / 128.0).astype(np.float16),
                            (128, OUT_FT)).copy()
    w3 = np.ascontiguousarray(
        W.reshape(2, 128, OUT_FT).transpose(1, 0, 2)).astype(
            ml_dtypes.bfloat16)  # [128, 2, OUT_FT]

    in_maps = []
    for k in range(NCORES):
        shard = np.zeros((NSP, IN_FT), np.float32)
        shard[:NS] = seq[k * NS:(k + 1) * NS]
        seqT_k = np.ascontiguousarray(shard.T).reshape(2, 128, NSP).astype(
            ml_dtypes.bfloat16)
        in_maps.append({
            "seqT": seqT_k,
            "gidx": gidx_full[k],
            "val": np.ascontiguousarray(valp[k]),
            "rl": np.ascontiguousarray(rlp[k]),
            "w": w3,
            "biasb": biasb,
        })
    return in_maps, lay


_PROGRAMS: dict[tuple, object] = {}


def kernel(seq, edge_row, edge_col, edge_val, W, b):
    in_maps, lay = prepare_inputs(seq, edge_row, edge_col, edge_val, W, b)
    prog = _PROGRAMS.get(lay.key())
    if prog is None:
        prog = _PROGRAMS[lay.key()] = build_program(lay)
    res = run_bass_kernel_spmd(prog, in_maps, core_ids=list(range(NCORES)))

    def unshard(name):
        # [128, NB, OUT_FT] partition-major -> [NS, OUT_FT] row-major
        parts = [
            res.results[k][name].transpose(1, 0, 2).reshape(NSP, OUT_FT)[:NS]
            for k in range(NCORES)
        ]
        return np.concatenate(parts)[None]

    return unshard("agg"), unshard("sf")


# revision 22
# speedup vs baseline: 1.3347x; 1.0742x over previous
"""GCN layer (dense projection + sparse neighbor aggregation) on 8 Trainium2
NeuronCores via Bass/Tile.

Strategy: shard nodes (and their incident edges, grouped by destination row)
across the 8 cores; replicate W/b; AllGather the projected node features so
every core can gather arbitrary source columns (split into 4 bucket-aligned
sub-collectives pipelined with the projection); per 128-row output block,
bulk-gather the needed source rows with DMAGatherAnt (int16 indices into 4
sub-tables of <=32k rows, ONE gather instruction per (group, bucket) to
amortize the ~1us SWDGE fixed cost), scale by edge_val, and segment-sum via
an assignment-matrix matmul accumulated in PSUM (bias folded in as an extra
rank-128 matmul; padded gather slots are killed by rowloc=-1).

Chunk capacities are per-(block, bucket) (max over the 8 cores only), cutting
gather padding vs. a single global capacity.
"""

import sys

if "/opt/trn_rl_repo" not in sys.path:
    sys.path.insert(0, "/opt/trn_rl_repo")

import numpy as np

import concourse.bass as bass
import concourse.mybir as mybir
import concourse.tile as tile
from concourse import bacc
from concourse.bass_utils import run_bass_kernel_spmd

N_NODES = 100000
N_EDGES = 1600000
IN_FT = 256
OUT_FT = 64
NCORES = 8
NS = N_NODES // NCORES          # 12500 nodes per core
NB = (NS + 127) // 128          # 98 row blocks per core
NSP = NB * 128                  # 12544 padded nodes per core
GB = 7                          # row blocks per pipeline group (98 = 14 * 7)
NGROUPS = NB // GB              # 14
QGROUPS = [2, 4, 4, 4]          # groups per sub-collective (sums to 14)
NSUB = len(QGROUPS)

F32 = mybir.dt.float32
F16 = mybir.dt.float16
BF16 = mybir.dt.bfloat16
I32 = mybir.dt.int32
I16 = mybir.dt.int16

NQ = 4                          # SWDGE queues
MAXI = 1024                     # max indices per dma_gather (HW/ucode cap)


def _quarter_layout():
    qg_end = np.cumsum(QGROUPS)                  # groups per bucket, cumul
    qb = [g * GB for g in QGROUPS]               # blocks per bucket
    qb_start = np.concatenate([[0], np.cumsum(qb)])
    subrows = [NCORES * 128 * n for n in qb]
    return qg_end, qb, qb_start, subrows


class Layout:
    """Derived index layout shared by host prep and program build.

    caps[j][b]: 128-edge chunk capacity of (dest block j, source bucket b).
    """

    def __init__(self, caps):
        self.caps = caps = np.asarray(caps)          # [NB, NSUB]
        qg_end, qb, qb_start, subrows = _quarter_layout()
        self.qg_end, self.qb, self.qb_start, self.subrows = (
            qg_end, qb, qb_start, subrows)
        self.ncht = caps.sum(axis=1)                 # chunks per block
        self.maxncht = int(self.ncht.max())
        # per-group stream: bucket-major regions, block-major within bucket
        self.sgb = np.zeros((NGROUPS, NSUB), np.int64)
        for g in range(NGROUPS):
            for b in range(NSUB):
                self.sgb[g, b] = caps[g * GB:(g + 1) * GB, b].sum()
        self.sgt = self.sgb.sum(axis=1)              # chunks per group
        # region offset of bucket b within group g's stream
        self.reg = np.zeros((NGROUPS, NSUB + 1), np.int64)
        self.reg[:, 1:] = np.cumsum(self.sgb, axis=1)
        # offset of block j's chunks within (group, bucket) region
        self.blkoff = np.zeros((NGROUPS, NSUB, GB + 1), np.int64)
        for g in range(NGROUPS):
            for b in range(NSUB):
                self.blkoff[g, b, 1:] = np.cumsum(
                    caps[g * GB:(g + 1) * GB, b])
        # flat DRAM offsets
        self.g_off = np.concatenate([[0], np.cumsum(self.sgt)])  # val cols
        self.total_sgt = int(self.g_off[-1])
        self.rl_off = np.concatenate([[0], np.cumsum(self.ncht)])  # rl cols
        self.total_ncht = int(self.rl_off[-1])

    def key(self):
        return tuple(map(int, self.caps.reshape(-1)))


def build_program(lay: Layout):
    """One SPMD Bass program; all 8 cores run it on their own shards."""
    caps, reg, blkoff, sgb = lay.caps, lay.reg, lay.blkoff, lay.sgb
    qg_end, qb, subrows = lay.qg_end, lay.qb, lay.subrows

    nc = bacc.Bacc("TRN2", target_bir_lowering=False, debug=False,
                   num_devices=NCORES, num_swdge_queues=NQ)

    seqT = nc.dram_tensor("seqT", [2, 128, NSP], BF16, kind="ExternalInput")
    gidx = nc.dram_tensor("gidx", [128, lay.total_sgt * 8], I16,
                          kind="ExternalInput")
    val = nc.dram_tensor("val", [128, lay.total_sgt], F16,
                         kind="ExternalInput")
    rl = nc.dram_tensor("rl", [128, lay.total_ncht], F16,
                        kind="ExternalInput")
    w_in = nc.dram_tensor("w", [128, 2, OUT_FT], BF16, kind="ExternalInput")
    bias_in = nc.dram_tensor("biasb", [128, OUT_FT], F16,
                             kind="ExternalInput")
    # partition-major layouts: [p, block, feature]; host un-permutes
    sf_out = nc.dram_tensor("sf", [128, NB, OUT_FT], F32,
                            kind="ExternalOutput")
    agg_out = nc.dram_tensor("agg", [128, NB, OUT_FT], F32,
                             kind="ExternalOutput")
    ccin = [nc.dram_tensor(f"ccin{b}", [128, qb[b], OUT_FT], F32)
            for b in range(NSUB)]
    xfull = [nc.dram_tensor(f"xfull{b}", [subrows[b], OUT_FT], F32,
                            addr_space="Shared") for b in range(NSUB)]

    groups = [list(range(NCORES))]

    with tile.TileContext(nc) as tc:
        with (
            tc.tile_pool(name="const", bufs=1) as cpool,
            tc.tile_pool(name="psum", bufs=2, space="PSUM") as psum_pool,
        ):
            w_sb = cpool.tile([128, 2, OUT_FT], BF16)
            nc.sync.dma_start(out=w_sb[:], in_=w_in[:])
            # bias/128 broadcast, fp16: added into PSUM via a ones matmul
            bias_sb = cpool.tile([128, OUT_FT], F16)
            nc.sync.dma_start(out=bias_sb[:], in_=bias_in[:])
            ones_sb = cpool.tile([128, 128], F16)
            nc.gpsimd.memset(ones_sb[:], 1.0)
            # iota-tiled constant: col (c*128 + q) = q, f16
            iota_i = cpool.tile([128, lay.maxncht, 128], I32)
            nc.gpsimd.iota(iota_i[:], pattern=[[0, lay.maxncht], [1, 128]],
                           base=0, channel_multiplier=0)
            iota_f = cpool.tile([128, lay.maxncht, 128], F16)
            nc.vector.tensor_copy(out=iota_f[:], in_=iota_i[:])

            # ---- phase 1: x = seq @ W (fp32) + bucket AllGathers ----
            with (
                tc.tile_pool(name="seqpool", bufs=1) as seqpool,
                tc.tile_pool(name="p1work", bufs=3) as p1work,
            ):
                seqT_sb = seqpool.tile([128, 2, NSP], BF16)
                # panel loads so early matmuls start promptly
                PAN = 2 * GB * 128          # 2 groups per panel
                for kc in range(2):
                    for p0 in range(0, NSP, PAN):
                        p1 = min(NSP, p0 + PAN)
                        nc.sync.dma_start(out=seqT_sb[:, kc, p0:p1],
                                          in_=seqT[kc, :, p0:p1])

                # index/val/rowloc loads queued after the seq panels: they
                # complete during the projection/AllGather window without
                # delaying phase 1 or competing with the gather drain
                gidx_sb = cpool.tile([128, lay.total_sgt * 8], I16)
                nc.sync.dma_start(out=gidx_sb[:], in_=gidx[:])
                val_sb = cpool.tile([128, lay.total_sgt], F16)
                nc.sync.dma_start(out=val_sb[:], in_=val[:])
                rl_sb = cpool.tile([128, lay.total_ncht], F16)
                nc.sync.dma_start(out=rl_sb[:], in_=rl[:])

                q = 0
                for g in range(NGROUPS):
                    x_sb = p1work.tile([128, GB, OUT_FT], F32, tag="x_sb")
                    for j in range(GB):
                        nb = g * GB + j
                        px = psum_pool.tile([128, OUT_FT], F32, tag="px")
                        for kc in range(2):
                            nc.tensor.matmul(
                                px[:],
                                seqT_sb[:, kc, nb * 128:(nb + 1) * 128],
                                w_sb[:, kc, :],
                                start=(kc == 0),
                                stop=(kc == 1),
                            )
                        nc.vector.tensor_copy(out=x_sb[:, j, :], in_=px[:])
                    nc.sync.dma_start(
                        out=sf_out[:, g * GB:(g + 1) * GB, :], in_=x_sb[:])
                    gq = g - (int(qg_end[q - 1]) if q else 0)
                    nc.sync.dma_start(
                        out=ccin[q][:, gq * GB:(gq + 1) * GB, :], in_=x_sb[:])
                    if g + 1 == qg_end[q]:
                        nc.gpsimd.collective_compute(
                            "AllGather",
                            mybir.AluOpType.bypass,
                            replica_groups=groups,
                            ins=[ccin[q][:]],
                            outs=[xfull[q][:]],
                        )
                        q += 1

            # ---- phase 2: bulk gather + scale + segment-sum matmul ----
            gq_ctr = [0]
            with (
                tc.tile_pool(name="p2work", bufs=2) as p2,
                tc.tile_pool(name="p2xg", bufs=2) as p2xg,
            ):
                for g in range(NGROUPS):
                    sgt_g = int(lay.sgt[g])
                    go = int(lay.g_off[g])
                    # xg chunk layout per group: bucket-major regions,
                    # block-major within bucket
                    xg = p2xg.tile([128, sgt_g, OUT_FT], F32, tag="xg")
                    xg16 = p2.tile([128, sgt_g, OUT_FT], F16, tag="xg16")
                    for b in range(NSUB):
                        nch = int(sgb[g, b])
                        if nch == 0:
                            continue
                        # split so one instruction's descriptors fit the ring
                        npc = -(-nch * 128 // MAXI)       # pieces
                        per = -(-nch // npc)              # chunks per piece
                        for off in range(0, nch, per):
                            ln = min(per, nch - off)
                            r0 = int(reg[g, b]) + off
                            nc.gpsimd.dma_gather(
                                out_ap=xg[:, r0:r0 + ln, :],
                                in_ap=xfull[b][:],
                                idxs_ap=gidx_sb[:, (go + r0) * 8:
                                                (go + r0 + ln) * 8],
                                num_idxs=ln * 128,
                                num_idxs_reg=ln * 128,
                                elem_size=OUT_FT,
                                queue_num=gq_ctr[0] % NQ,
                            )
                            gq_ctr[0] += 1
                        # fold edge_val in while casting f32 -> fp16, per
                        # bucket region so xg frees early and the last
                        # group's tail is short
                        r0 = int(reg[g, b])
                        nc.vector.tensor_tensor(
                            out=xg16[:, r0:r0 + nch, :],
                            in0=xg[:, r0:r0 + nch, :],
                            in1=val_sb[:, go + r0:go + r0 + nch].unsqueeze(
                                2).broadcast_to([128, nch, OUT_FT]),
                            op=mybir.AluOpType.mult,
                        )
                    out_sb = p2.tile([128, GB, OUT_FT], F32, tag="out_sb")
                    for j in range(GB):
                        jg = g * GB + j
                        ncht_j = int(lay.ncht[jg])
                        # A[p, c, q] = (rowloc[p, c] == q); -1 pads vanish
                        a_sb = p2.tile([128, lay.maxncht * 128], F16,
                                       tag="a_sb")
                        c0 = int(lay.rl_off[jg])
                        nc.vector.tensor_tensor(
                            out=a_sb[:, :ncht_j * 128].rearrange(
                                "p (c q) -> p c q", q=128),
                            in0=iota_f[:, :ncht_j, :],
                            in1=rl_sb[:, c0:c0 + ncht_j].unsqueeze(
                                2).broadcast_to([128, ncht_j, 128]),
                            op=mybir.AluOpType.is_equal,
                        )
                        po = psum_pool.tile([128, OUT_FT], F32, tag="po")
                        nc.tensor.matmul(po[:], ones_sb[:], bias_sb[:],
                                         start=True, stop=False)
                        ci = 0
                        for b in range(NSUB):
                            nchjb = int(caps[jg, b])
                            rcb = int(reg[g, b] + blkoff[g, b, j])
                            for cc in range(nchjb):
                                nc.tensor.matmul(
                                    po[:],
                                    a_sb[:, (ci + cc) * 128:
                                         (ci + cc + 1) * 128],
                                    xg16[:, rcb + cc, :],
                                    start=False,
                                    stop=(ci + cc == ncht_j - 1),
                                )
                            ci += nchjb
                        nc.scalar.activation(
                            out=out_sb[:, j, :], in_=po[:],
                            func=mybir.ActivationFunctionType.Relu)
                    nc.sync.dma_start(
                        out=agg_out[:, g * GB:(g + 1) * GB, :], in_=out_sb[:])

    nc.compile()
    return nc


def prepare_inputs(seq, edge_row, edge_col, edge_val, W, b):
    """Host-side sharding / graph partitioning. Returns (in_maps, layout)."""
    seq = np.asarray(seq, dtype=np.float32).reshape(N_NODES, IN_FT)
    r = np.asarray(edge_row).astype(np.int64)
    c = np.asarray(edge_col).astype(np.int64)
    v = np.asarray(edge_val, dtype=np.float32)
    W = np.asarray(W, dtype=np.float32).reshape(IN_FT, OUT_FT)
    b = np.asarray(b, dtype=np.float32).reshape(OUT_FT)

    qg_end, qb, qb_start, subrows = _quarter_layout()
    qb_start = qb_start.astype(np.int64)
    # bucket of each block index
    blk_q = np.searchsorted(qb_start[1:], np.arange(NB), side="right")

    # feature-table row within its bucket sub-table (core, partition, block)
    csrc = c // NS
    crem = c % NS
    cblk = crem // 128
    cp = crem % 128
    cq = blk_q[cblk]
    nqb = np.asarray(qb)[cq]
    lidx = (csrc * 128 * nqb + cp * nqb + (cblk - qb_start[cq])).astype(
        np.int16)

    core = r // NS
    loc = r - core * NS
    blk = loc >> 7
    rowloc = (loc & 127).astype(np.float16)
    bucket = cq

    # per-(block, bucket) chunk capacity: max count over the 8 cores
    key = (core * NB + blk) * NSUB + bucket
    ngrp = NCORES * NB * NSUB
    counts = np.bincount(key, minlength=ngrp).reshape(NCORES, NB, NSUB)
    caps = np.ceil(counts.max(axis=0) / 128).astype(np.int64)  # [NB, NSUB]
    caps = np.maximum(caps, 1)
    lay = Layout(caps)

    # edge destination slot within the flat per-core stream; edges within a
    # (core, block, bucket) run sorted by source row for HBM read locality
    order = np.lexsort((lidx, key))
    key_s = key[order]
    starts = np.searchsorted(key_s, np.arange(ngrp))
    pos = np.arange(N_EDGES) - starts[key_s]           # rank within (c,j,b)
    kb = key_s % NSUB
    kj = (key_s // NSUB) % NB
    kcore = key_s // (NSUB * NB)
    kg = kj // GB
    kjl = kj % GB
    # chunk column within the group stream; lane within chunk
    sc = (lay.reg[kg, kb] + lay.blkoff[kg, kb, kjl] + pos // 128)
    lane = pos % 128
    # global chunk col across groups (flat val layout)
    gchunk = lay.g_off[kg] + sc
    # rl col within flat rl layout
    capcum = np.zeros((NB, NSUB + 1), np.int64)
    capcum[:, 1:] = np.cumsum(caps, axis=1)
    rlcol = lay.rl_off[kj] + capcum[kj, kb] + pos // 128

    tot_chunks = lay.total_sgt
    valp = np.zeros((NCORES, 128, tot_chunks), np.float16)
    idxp = np.zeros((NCORES, 128, tot_chunks), np.int16)   # pad: row 0
    rlp = np.full((NCORES, 128, lay.total_ncht), -1.0, np.float16)
    valp[kcore, lane, gchunk] = v[order].astype(np.float16)
    idxp[kcore, lane, gchunk] = lidx[order]
    rlp[kcore, lane, rlcol] = rowloc[order]

    # gidx 16-wrap per group: idx stream i -> [i % 16, i // 16], x8 replicate
    gidx_arr = np.empty((NCORES, 16, tot_chunks * 8), np.int16)
    for g in range(NGROUPS):
        s0, s1 = int(lay.g_off[g]), int(lay.g_off[g + 1])
        seg = idxp[:, :, s0:s1]                      # [NCORES, 128lane, sg]
        # stream order: chunk-major, lane-minor -> i = sc*128 + lane
        stream = seg.transpose(0, 2, 1).reshape(NCORES, (s1 - s0) * 128)
        wi = stream.reshape(NCORES, (s1 - s0) * 8, 16)
        gidx_arr[:, :, s0 * 8:s1 * 8] = wi.transpose(0, 2, 1)
    gidx_full = np.broadcast_to(
        gidx_arr[:, None], (NCORES, 8, 16, tot_chunks * 8))
    gidx_full = np.ascontiguousarray(
        gidx_full.reshape(NCORES, 128, tot_chunks * 8))

    import ml_dtypes

    biasb = np.broadcast_to((b / 128.0).astype(np.float16),
                            (128, OUT_FT)).copy()
    w3 = np.ascontiguousarray(
        W.reshape(2, 128, OUT_FT).transpose(1, 0, 2)).astype(
            ml_dtypes.bfloat16)  # [128, 2, OUT_FT]

    in_maps = []
    for k in range(NCORES):
        shard = np.zeros((NSP, IN_FT), np.float32)
        shard[:NS] = seq[k * NS:(k + 1) * NS]
        seqT_k = np.ascontiguousarray(shard.T).reshape(2, 128, NSP).astype(
            ml_dtypes.bfloat16)
        in_maps.append({
            "seqT": seqT_k,
            "gidx": gidx_full[k],
            "val": np.ascontiguousarray(valp[k]),
            "rl": np.ascontiguousarray(rlp[k]),
            "w": w3,
            "biasb": biasb,
        })
    return in_maps, lay


_PROGRAMS: dict[tuple, object] = {}


def kernel(seq, edge_row, edge_col, edge_val, W, b):
    in_maps, lay = prepare_inputs(seq, edge_row, edge_col, edge_val, W, b)
    prog = _PROGRAMS.get(lay.key())
    if prog is None:
        prog = _PROGRAMS[lay.key()] = build_program(lay)
    res = run_bass_kernel_spmd(prog, in_maps, core_ids=list(range(NCORES)))

    def unshard(name):
        # [128, NB, OUT_FT] partition-major -> [NS, OUT_FT] row-major
        parts = [
            res.results[k][name].transpose(1, 0, 2).reshape(NSP, OUT_FT)[:NS]
            for k in range(NCORES)
        ]
        return np.concatenate(parts)[None]

    return unshard("agg"), unshard("sf")


# revision 28
# speedup vs baseline: 1.3603x; 1.0192x over previous
"""GCN layer (dense projection + sparse neighbor aggregation) on 8 Trainium2
NeuronCores via Bass/Tile.

Strategy: shard nodes (and their incident edges, grouped by destination row)
across the 8 cores; replicate W/b; AllGather the projected node features so
every core can gather arbitrary source columns (split into 4 bucket-aligned
sub-collectives pipelined with the projection); per 128-row output block,
bulk-gather the needed source rows with DMAGatherAnt (int16 indices into 4
sub-tables of <=32k rows, ONE gather instruction per (group, bucket) to
amortize the ~1us SWDGE fixed cost), scale by edge_val, and segment-sum via
an assignment-matrix matmul accumulated in PSUM (bias folded in as an extra
rank-128 matmul; padded gather slots are killed by rowloc=-1).

Chunk capacities are per-(block, bucket) (max over the 8 cores only), cutting
gather padding vs. a single global capacity.
"""

import sys

if "/opt/trn_rl_repo" not in sys.path:
    sys.path.insert(0, "/opt/trn_rl_repo")

import numpy as np

import concourse.bass as bass
import concourse.mybir as mybir
import concourse.tile as tile
from concourse import bacc
from concourse.bass_utils import run_bass_kernel_spmd

N_NODES = 100000
N_EDGES = 1600000
IN_FT = 256
OUT_FT = 64
NCORES = 8
NS = N_NODES // NCORES          # 12500 nodes per core
NB = (NS + 127) // 128          # 98 row blocks per core
NSP = NB * 128                  # 12544 padded nodes per core
GB = 7                          # row blocks per pipeline group (98 = 14 * 7)
NGROUPS = NB // GB              # 14
QGROUPS = [2, 4, 4, 4]          # groups per sub-collective (sums to 14)
NSUB = len(QGROUPS)

F32 = mybir.dt.float32
F16 = mybir.dt.float16
BF16 = mybir.dt.bfloat16
I32 = mybir.dt.int32
I16 = mybir.dt.int16

NQ = 4                          # SWDGE queues
MAXI = 1024                     # max indices per dma_gather (HW/ucode cap)


def _quarter_layout():
    qg_end = np.cumsum(QGROUPS)                  # groups per bucket, cumul
    qb = [g * GB for g in QGROUPS]               # blocks per bucket
    qb_start = np.concatenate([[0], np.cumsum(qb)])
    subrows = [NCORES * 128 * n for n in qb]
    return qg_end, qb, qb_start, subrows


class Layout:
    """Derived index layout shared by host prep and program build.

    caps[j][b]: 128-edge chunk capacity of (dest block j, source bucket b).
    """

    def __init__(self, caps):
        self.caps = caps = np.asarray(caps)          # [NB, NSUB]
        qg_end, qb, qb_start, subrows = _quarter_layout()
        self.qg_end, self.qb, self.qb_start, self.subrows = (
            qg_end, qb, qb_start, subrows)
        self.ncht = caps.sum(axis=1)                 # chunks per block
        self.maxncht = int(self.ncht.max())
        # per-group stream: bucket-major regions, block-major within bucket
        self.sgb = np.zeros((NGROUPS, NSUB), np.int64)
        for g in range(NGROUPS):
            for b in range(NSUB):
                self.sgb[g, b] = caps[g * GB:(g + 1) * GB, b].sum()
        self.sgt = self.sgb.sum(axis=1)              # chunks per group
        # region offset of bucket b within group g's stream
        self.reg = np.zeros((NGROUPS, NSUB + 1), np.int64)
        self.reg[:, 1:] = np.cumsum(self.sgb, axis=1)
        # offset of block j's chunks within (group, bucket) region
        self.blkoff = np.zeros((NGROUPS, NSUB, GB + 1), np.int64)
        for g in range(NGROUPS):
            for b in range(NSUB):
                self.blkoff[g, b, 1:] = np.cumsum(
                    caps[g * GB:(g + 1) * GB, b])
        # flat DRAM offsets
        self.g_off = np.concatenate([[0], np.cumsum(self.sgt)])  # val cols
        self.total_sgt = int(self.g_off[-1])
        self.rl_off = np.concatenate([[0], np.cumsum(self.ncht)])  # rl cols
        self.total_ncht = int(self.rl_off[-1])

    def key(self):
        return tuple(map(int, self.caps.reshape(-1)))


def build_program(lay: Layout):
    """One SPMD Bass program; all 8 cores run it on their own shards."""
    caps, reg, blkoff, sgb = lay.caps, lay.reg, lay.blkoff, lay.sgb
    qg_end, qb, subrows = lay.qg_end, lay.qb, lay.subrows

    nc = bacc.Bacc("TRN2", target_bir_lowering=False, debug=False,
                   num_devices=NCORES, num_swdge_queues=NQ)

    seqT = nc.dram_tensor("seqT", [2, 128, NSP], BF16, kind="ExternalInput")
    gidx = nc.dram_tensor("gidx", [128, lay.total_sgt * 8], I16,
                          kind="ExternalInput")
    # edge_val pre-split by source-row parity: valE picks even rows of the
    # gathered bf16 pair, valO odd rows (the other half is zeroed)
    valE = nc.dram_tensor("valE", [128, lay.total_sgt], BF16,
                          kind="ExternalInput")
    valO = nc.dram_tensor("valO", [128, lay.total_sgt], BF16,
                          kind="ExternalInput")
    rl = nc.dram_tensor("rl", [128, lay.total_ncht], BF16,
                        kind="ExternalInput")
    w_in = nc.dram_tensor("w", [128, 2, OUT_FT], BF16, kind="ExternalInput")
    bias_in = nc.dram_tensor("biasb", [128, OUT_FT], BF16,
                             kind="ExternalInput")
    # partition-major layouts: [p, block, feature]; host un-permutes
    sf_out = nc.dram_tensor("sf", [128, NB, OUT_FT], F32,
                            kind="ExternalOutput")
    agg_out = nc.dram_tensor("agg", [128, NB, OUT_FT], F32,
                             kind="ExternalOutput")
    ccin = [nc.dram_tensor(f"ccin{b}", [128, qb[b], OUT_FT], BF16)
            for b in range(NSUB)]
    # bf16 feature tables, viewed as row PAIRS (256B granule for the gather)
    xfull = [nc.dram_tensor(f"xfull{b}", [subrows[b] // 2, 2 * OUT_FT], BF16,
                            addr_space="Shared") for b in range(NSUB)]

    groups = [list(range(NCORES))]

    with tile.TileContext(nc) as tc:
        with (
            tc.tile_pool(name="const", bufs=1) as cpool,
            tc.tile_pool(name="psum", bufs=2, space="PSUM") as psum_pool,
        ):
            w_sb = cpool.tile([128, 2, OUT_FT], BF16)
            nc.sync.dma_start(out=w_sb[:], in_=w_in[:])
            # bias/128 broadcast: added into PSUM via a ones matmul
            bias_sb = cpool.tile([128, OUT_FT], BF16)
            nc.sync.dma_start(out=bias_sb[:], in_=bias_in[:])
            ones_sb = cpool.tile([128, 128], BF16)
            nc.gpsimd.memset(ones_sb[:], 1.0)
            # iota-tiled constant: col (c*128 + q) = q
            iota_i = cpool.tile([128, lay.maxncht, 128], I32)
            nc.gpsimd.iota(iota_i[:], pattern=[[0, lay.maxncht], [1, 128]],
                           base=0, channel_multiplier=0)
            iota_f = cpool.tile([128, lay.maxncht, 128], BF16)
            nc.vector.tensor_copy(out=iota_f[:], in_=iota_i[:])

            # ---- phase 1: x = seq @ W (fp32) + bucket AllGathers ----
            with (
                tc.tile_pool(name="seqpool", bufs=1) as seqpool,
                tc.tile_pool(name="p1work", bufs=3) as p1work,
            ):
                seqT_sb = seqpool.tile([128, 2, NSP], BF16)
                # panel loads so early matmuls start promptly
                PAN = 2 * GB * 128          # 2 groups per panel
                for kc in range(2):
                    for p0 in range(0, NSP, PAN):
                        p1 = min(NSP, p0 + PAN)
                        nc.sync.dma_start(out=seqT_sb[:, kc, p0:p1],
                                          in_=seqT[kc, :, p0:p1])

                # index/val/rowloc loads queued after the seq panels: they
                # complete during the projection/AllGather window without
                # delaying phase 1 or competing with the gather drain
                gidx_sb = cpool.tile([128, lay.total_sgt * 8], I16)
                nc.sync.dma_start(out=gidx_sb[:], in_=gidx[:])
                valE_sb = cpool.tile([128, lay.total_sgt], BF16)
                nc.sync.dma_start(out=valE_sb[:], in_=valE[:])
                valO_sb = cpool.tile([128, lay.total_sgt], BF16)
                nc.sync.dma_start(out=valO_sb[:], in_=valO[:])
                rl_sb = cpool.tile([128, lay.total_ncht], BF16)
                nc.sync.dma_start(out=rl_sb[:], in_=rl[:])

                q = 0
                for g in range(NGROUPS):
                    x_sb = p1work.tile([128, GB, OUT_FT], F32, tag="x_sb")
                    xb_sb = p1work.tile([128, GB, OUT_FT], BF16, tag="xb_sb")
                    for j in range(GB):
                        nb = g * GB + j
                        px = psum_pool.tile([128, OUT_FT], F32, tag="px")
                        for kc in range(2):
                            nc.tensor.matmul(
                                px[:],
                                seqT_sb[:, kc, nb * 128:(nb + 1) * 128],
                                w_sb[:, kc, :],
                                start=(kc == 0),
                                stop=(kc == 1),
                            )
                        nc.vector.tensor_copy(out=x_sb[:, j, :], in_=px[:])
                        nc.scalar.activation(
                            out=xb_sb[:, j, :], in_=px[:],
                            func=mybir.ActivationFunctionType.Copy)
                    nc.sync.dma_start(
                        out=sf_out[:, g * GB:(g + 1) * GB, :], in_=x_sb[:])
                    gq = g - (int(qg_end[q - 1]) if q else 0)
                    nc.sync.dma_start(
                        out=ccin[q][:, gq * GB:(gq + 1) * GB, :], in_=xb_sb[:])
                    if g + 1 == qg_end[q]:
                        nc.gpsimd.collective_compute(
                            "AllGather",
                            mybir.AluOpType.bypass,
                            replica_groups=groups,
                            ins=[ccin[q][:]],
                            outs=[xfull[q][:]],
                        )
                        q += 1

            # ---- phase 2: bulk gather + scale + segment-sum matmul ----
            gq_ctr = [0]
            with (
                tc.tile_pool(name="p2work", bufs=2) as p2,
                tc.tile_pool(name="p2xg", bufs=2) as p2xg,
            ):
                for g in range(NGROUPS):
                    sgt_g = int(lay.sgt[g])
                    go = int(lay.g_off[g])
                    # xg chunk layout per group: bucket-major regions,
                    # block-major within bucket; each slot holds a bf16 row
                    # PAIR (even row in [0:64], odd row in [64:128])
                    xg = p2xg.tile([128, sgt_g, 2 * OUT_FT], BF16, tag="xg")
                    for b in range(NSUB):
                        nch = int(sgb[g, b])
                        if nch == 0:
                            continue
                        # split so one instruction's descriptors fit the ring
                        npc = -(-nch * 128 // MAXI)       # pieces
                        per = -(-nch // npc)              # chunks per piece
                        for off in range(0, nch, per):
                            ln = min(per, nch - off)
                            r0 = int(reg[g, b]) + off
                            nc.gpsimd.dma_gather(
                                out_ap=xg[:, r0:r0 + ln, :],
                                in_ap=xfull[b][:],
                                idxs_ap=gidx_sb[:, (go + r0) * 8:
                                                (go + r0 + ln) * 8],
                                num_idxs=ln * 128,
                                num_idxs_reg=ln * 128,
                                elem_size=2 * OUT_FT,
                                queue_num=gq_ctr[0] % NQ,
                            )
                            gq_ctr[0] += 1
                        # fold edge_val in, in place, per pair half: the
                        # half not selected by the edge is scaled by 0
                        r0 = int(reg[g, b])
                        nc.vector.tensor_tensor(
                            out=xg[:, r0:r0 + nch, 0:OUT_FT],
                            in0=xg[:, r0:r0 + nch, 0:OUT_FT],
                            in1=valE_sb[:, go + r0:go + r0 + nch].unsqueeze(
                                2).broadcast_to([128, nch, OUT_FT]),
                            op=mybir.AluOpType.mult,
                        )
                        nc.vector.tensor_tensor(
                            out=xg[:, r0:r0 + nch, OUT_FT:2 * OUT_FT],
                            in0=xg[:, r0:r0 + nch, OUT_FT:2 * OUT_FT],
                            in1=valO_sb[:, go + r0:go + r0 + nch].unsqueeze(
                                2).broadcast_to([128, nch, OUT_FT]),
                            op=mybir.AluOpType.mult,
                        )
                    out_sb = p2.tile([128, GB, OUT_FT], F32, tag="out_sb")
                    for j in range(GB):
                        jg = g * GB + j
                        ncht_j = int(lay.ncht[jg])
                        # A[p, c, q] = (rowloc[p, c] == q); -1 pads vanish
                        a_sb = p2.tile([128, lay.maxncht * 128], BF16,
                                       tag="a_sb")
                        c0 = int(lay.rl_off[jg])
                        nc.vector.tensor_tensor(
                            out=a_sb[:, :ncht_j * 128].rearrange(
                                "p (c q) -> p c q", q=128),
                            in0=iota_f[:, :ncht_j, :],
                            in1=rl_sb[:, c0:c0 + ncht_j].unsqueeze(
                                2).broadcast_to([128, ncht_j, 128]),
                            op=mybir.AluOpType.is_equal,
                        )
                        po = psum_pool.tile([128, OUT_FT], F32, tag="po")
                        nc.tensor.matmul(po[:], ones_sb[:], bias_sb[:],
                                         start=True, stop=False)
                        ci = 0
                        for b in range(NSUB):
                            nchjb = int(caps[jg, b])
                            rcb = int(reg[g, b] + blkoff[g, b, j])
                            for cc in range(nchjb):
                                a_c = a_sb[:, (ci + cc) * 128:
                                           (ci + cc + 1) * 128]
                                last = ci + cc == ncht_j - 1
                                nc.tensor.matmul(
                                    po[:], a_c,
                                    xg[:, rcb + cc, 0:OUT_FT],
                                    start=False, stop=False,
                                )
                                nc.tensor.matmul(
                                    po[:], a_c,
                                    xg[:, rcb + cc, OUT_FT:2 * OUT_FT],
                                    start=False, stop=last,
                                )
                            ci += nchjb
                        nc.scalar.activation(
                            out=out_sb[:, j, :], in_=po[:],
                            func=mybir.ActivationFunctionType.Relu)
                    nc.sync.dma_start(
                        out=agg_out[:, g * GB:(g + 1) * GB, :], in_=out_sb[:])

    nc.compile()
    return nc


def prepare_inputs(seq, edge_row, edge_col, edge_val, W, b):
    """Host-side sharding / graph partitioning. Returns (in_maps, layout)."""
    seq = np.asarray(seq, dtype=np.float32).reshape(N_NODES, IN_FT)
    r = np.asarray(edge_row).astype(np.int64)
    c = np.asarray(edge_col).astype(np.int64)
    v = np.asarray(edge_val, dtype=np.float32)
    W = np.asarray(W, dtype=np.float32).reshape(IN_FT, OUT_FT)
    b = np.asarray(b, dtype=np.float32).reshape(OUT_FT)

    qg_end, qb, qb_start, subrows = _quarter_layout()
    qb_start = qb_start.astype(np.int64)
    # bucket of each block index
    blk_q = np.searchsorted(qb_start[1:], np.arange(NB), side="right")

    # feature-table row within its bucket sub-table (core, partition, block)
    csrc = c // NS
    crem = c % NS
    cblk = crem // 128
    cp = crem % 128
    cq = blk_q[cblk]
    nqb = np.asarray(qb)[cq]
    lidx = (csrc * 128 * nqb + cp * nqb + (cblk - qb_start[cq])).astype(
        np.int16)

    core = r // NS
    loc = r - core * NS
    blk = loc >> 7
    rowloc = (loc & 127).astype(np.float16)
    bucket = cq

    # per-(block, bucket) chunk capacity: max count over the 8 cores
    key = (core * NB + blk) * NSUB + bucket
    ngrp = NCORES * NB * NSUB
    counts = np.bincount(key, minlength=ngrp).reshape(NCORES, NB, NSUB)
    caps = np.ceil(counts.max(axis=0) / 128).astype(np.int64)  # [NB, NSUB]
    caps = np.maximum(caps, 1)
    lay = Layout(caps)

    # edge destination slot within the flat per-core stream; edges within a
    # (core, block, bucket) run sorted by source row for HBM read locality
    order = np.lexsort((lidx, key))
    key_s = key[order]
    starts = np.searchsorted(key_s, np.arange(ngrp))
    pos = np.arange(N_EDGES) - starts[key_s]           # rank within (c,j,b)
    kb = key_s % NSUB
    kj = (key_s // NSUB) % NB
    kcore = key_s // (NSUB * NB)
    kg = kj // GB
    kjl = kj % GB
    # chunk column within the group stream; lane within chunk
    sc = (lay.reg[kg, kb] + lay.blkoff[kg, kb, kjl] + pos // 128)
    lane = pos % 128
    # global chunk col across groups (flat val layout)
    gchunk = lay.g_off[kg] + sc
    # rl col within flat rl layout
    capcum = np.zeros((NB, NSUB + 1), np.int64)
    capcum[:, 1:] = np.cumsum(caps, axis=1)
    rlcol = lay.rl_off[kj] + capcum[kj, kb] + pos // 128

    import ml_dtypes

    tot_chunks = lay.total_sgt
    # pair granule: index = row >> 1; parity selects the 64-wide half
    lidx_s = lidx[order]
    half_s = (lidx_s & 1).astype(np.float32)
    v_s = v[order].astype(np.float32)
    valEp = np.zeros((NCORES, 128, tot_chunks), ml_dtypes.bfloat16)
    valOp = np.zeros((NCORES, 128, tot_chunks), ml_dtypes.bfloat16)
    idxp = np.zeros((NCORES, 128, tot_chunks), np.int16)   # pad: pair 0
    rlp = np.full((NCORES, 128, lay.total_ncht), -1.0, ml_dtypes.bfloat16)
    valEp[kcore, lane, gchunk] = (v_s * (1.0 - half_s)).astype(
        ml_dtypes.bfloat16)
    valOp[kcore, lane, gchunk] = (v_s * half_s).astype(ml_dtypes.bfloat16)
    idxp[kcore, lane, gchunk] = lidx_s >> 1
    rlp[kcore, lane, rlcol] = rowloc[order].astype(ml_dtypes.bfloat16)

    # gidx 16-wrap per group: idx stream i -> [i % 16, i // 16], x8 replicate
    gidx_arr = np.empty((NCORES, 16, tot_chunks * 8), np.int16)
    for g in range(NGROUPS):
        s0, s1 = int(lay.g_off[g]), int(lay.g_off[g + 1])
        seg = idxp[:, :, s0:s1]                      # [NCORES, 128lane, sg]
        # stream order: chunk-major, lane-minor -> i = sc*128 + lane
        stream = seg.transpose(0, 2, 1).reshape(NCORES, (s1 - s0) * 128)
        wi = stream.reshape(NCORES, (s1 - s0) * 8, 16)
        gidx_arr[:, :, s0 * 8:s1 * 8] = wi.transpose(0, 2, 1)
    gidx_full = np.broadcast_to(
        gidx_arr[:, None], (NCORES, 8, 16, tot_chunks * 8))
    gidx_full = np.ascontiguousarray(
        gidx_full.reshape(NCORES, 128, tot_chunks * 8))

    biasb = np.broadcast_to((b / 128.0).astype(ml_dtypes.bfloat16),
                            (128, OUT_FT)).copy()
    w3 = np.ascontiguousarray(
        W.reshape(2, 128, OUT_FT).transpose(1, 0, 2)).astype(
            ml_dtypes.bfloat16)  # [128, 2, OUT_FT]

    in_maps = []
    for k in range(NCORES):
        shard = np.zeros((NSP, IN_FT), np.float32)
        shard[:NS] = seq[k * NS:(k + 1) * NS]
        seqT_k = np.ascontiguousarray(shard.T).reshape(2, 128, NSP).astype(
            ml_dtypes.bfloat16)
        in_maps.append({
            "seqT": seqT_k,
            "gidx": gidx_full[k],
            "valE": np.ascontiguousarray(valEp[k]),
            "valO": np.ascontiguousarray(valOp[k]),
            "rl": np.ascontiguousarray(rlp[k]),
            "w": w3,
            "biasb": biasb,
        })
    return in_maps, lay


_PROGRAMS: dict[tuple, object] = {}


def kernel(seq, edge_row, edge_col, edge_val, W, b):
    in_maps, lay = prepare_inputs(seq, edge_row, edge_col, edge_val, W, b)
    prog = _PROGRAMS.get(lay.key())
    if prog is None:
        prog = _PROGRAMS[lay.key()] = build_program(lay)
    res = run_bass_kernel_spmd(prog, in_maps, core_ids=list(range(NCORES)))

    def unshard(name):
        # [128, NB, OUT_FT] partition-major -> [NS, OUT_FT] row-major
        parts = [
            res.results[k][name].transpose(1, 0, 2).reshape(NSP, OUT_FT)[:NS]
            for k in range(NCORES)
        ]
        return np.concatenate(parts)[None]

    return unshard("agg"), unshard("sf")
